# revision 15
# baseline (speedup 1.0000x reference)
"""AttentionBlock Trainium2 kernel (B=4, N=2048, C=1024, H=16, D=64, EMB=1024).

    se = emb @ W_emb.T + b_emb;  scale, shift = split(se, 2, -1)
    h  = LN(x) * (1+scale) + shift
    q,k,v = split(h @ W_proj.T) -> (B,H,N,D);  q = LN(q); k = LN(k)  (over D)
    o  = softmax(q k^T / sqrt(D)) v  -> (B,N,C)
    out = o + o @ W_out.T

Sharding: 8 cores; core c -> batch b=c//2, query-half j=c%2. The host rolls
the token axis per core so its query tokens are always tokens 0:1024
(attention is permutation-equivariant over key/value tokens), giving one
symmetric SPMD NEFF with no collectives. Each core computes the full-batch
preamble (se/h/k/v over all 2048 tokens), and q/attention/out-proj for its
1024 rows.

Dataflow is feature-major (channels on partitions) end to end:
  - LayerNorm over channels == partition reduction -> ones-column matmuls.
  - Per-token (free-dim) scalars broadcast across partitions by bouncing a
    row through DRAM (DRAM APs allow step-0 partition dims; SBUF APs don't).
  - q/k LN centering rides the score matmul as an augmented 65th row
    (k_aug row64 = 8*mu_k[m], q_aug row64 = -8*mu_q[n]*rq[n]); the rk[m]/8
    factor is applied by the ACT exp per-partition scale operand.
  - k-side stat row-matmuls share one accumulation group / PSUM tile:
    cols 0/64 of the 1/8-selector give 8*mu at rows 0/64, cols 1/65 of the
    1/64-selector give E[k^2] at rows 1/65.
  - Softmax denominators come free as a ones column appended to v; the
    division is deferred until after the attn@v matmul.
  - The residual is folded into the output projection: W_res = (I+W_out).T.

Perf structure (second generation of this kernel):
  - Scores run per (head, key-chunk) over the FULL 1024 query tokens so the
    exp consumes a 2-bank [128,1024] PSUM tile in one ACT instruction
    (amortizes ACT's ~440-cycle SBUF access latency), and the attn@v matmul
    is a single N=1024 bf16 instruction per key chunk.
  - rsqrt = exp(-0.5*ln(var+eps)) on ACT keeps every activation used (Exp,
    Ln, Copy/Identity) inside the one `natural_log_exp_and_others` table
    set: no ACT table reloads.
  - Softmax reciprocal via the 1-instruction DVE approx (~18 bits); the
    unnormalized o and the reciprocal row are staged out of PSUM so the
    accumulator bank frees immediately.
  - GPSIMD (no PSUM port) takes SBUF->SBUF f32 work: squares for the
    variance stats and the FiLM multiply-add tail. ACT takes per-partition
    bias adds and one of the two k PSUM->SBUF head copies.
  - bf16 for: emb/W_emb (se matmul), q/k score operands, attention
    probabilities and v, o_fm and W_res. f32r elsewhere.
  - Head-quad software pipelining: both head-pairs of a quad are projected
    before either one's score phase, so the q/k-stat latency chains hide
    under projection matmuls and the PE never idles long enough for the
    HAM clock gate to re-throttle.
"""

import sys

sys.path.insert(0, "/opt/trn_rl_repo")

import math
from types import SimpleNamespace

import ml_dtypes
import numpy as np

import concourse.bass as bass
import concourse.mybir as mybir
import concourse.tile as tile
from concourse import bacc
from concourse.bass_utils import run_bass_kernel_spmd

P = 128
B, N, C = 4, 2048, 1024
H, D = 16, 64
EMB = 1024
EPS = 1e-5
T = N          # tokens per batch on each core (k/v coverage)
TQ = N // 2    # query tokens per core
CH = C // P    # 8 channel chunks
O2 = 2 * C
NCORES = 8
TT = 512       # token tile in phase A1
NTT = T // TT  # 4
NMT = T // 512   # 4  key-token tiles (512)
NMC = T // P     # 16 key-token chunks (128)
LN8 = math.log(0.125)

F32 = mybir.dt.float32
F32R = mybir.dt.float32r
BF16 = mybir.dt.bfloat16
MUL = mybir.AluOpType.mult
ADD = mybir.AluOpType.add
SUB = mybir.AluOpType.subtract
EXP = mybir.ActivationFunctionType.Exp
LOGN = mybir.ActivationFunctionType.Ln

_cached = {}


def _consts(E):
    nc, const = E.nc, E.const
    E.eps_t = const.tile([P, 1], F32, name="eps_t")
    nc.vector.memset(E.eps_t[:], EPS)
    E.ln8_t = const.tile([P, 1], F32, name="ln8_t")
    nc.vector.memset(E.ln8_t[:], LN8)
    # memset can't emit float32r: stage constants in F32, copy-round.
    cscr = const.tile([P, 66], F32, name="cscr")
    E.ones_col = const.tile([P, 1], F32R, name="ones_col")
    nc.vector.memset(cscr[:, 0:1], 1.0)
    nc.vector.tensor_copy(E.ones_col[:], cscr[:, 0:1])
    # k row stats: one group, two matmuls. bo8k puts 8*mu at rows 0/64
    # (cols 0/64), bo64k puts E[k^2] at rows 1/65 (cols 1/65).
    E.bo8k = const.tile([P, 66], F32R, name="bo8k")
    E.bo64k = const.tile([P, 66], F32R, name="bo64k")
    # q row stats: separate tiles, rows 0/64 in each.
    E.bon8 = const.tile([P, 65], BF16, name="bon8")    # -1/8
    E.bo64q = const.tile([P, 65], F32R, name="bo64q")  # +1/64
    for t_, v_, c0, c1, w in ((E.bo8k, 0.125, 0, 64, 66),
                              (E.bo64k, 1.0 / 64, 1, 65, 66),
                              (E.bon8, -0.125, 0, 64, 65),
                              (E.bo64q, 1.0 / 64, 0, 64, 65)):
        nc.vector.memset(cscr[:], 0.0)
        nc.vector.memset(cscr[0:64, c0:c0 + 1], v_)
        nc.vector.memset(cscr[64:128, c1:c1 + 1], v_)
        nc.vector.tensor_copy(t_[:], cscr[:, 0:w])
    E.bemb_sb = const.tile([P, O2 // P], F32, name="bemb_sb")
    nc.sync.dma_start(E.bemb_sb[:], E.bemb[:])


def _a1_tile(E, pools, tt):
    """One 512-token tile of phase A1: stats, se matmuls, FiLM."""
    nc, dram = E.nc, E.dram
    a1s, a1r, ps_a1, ps_st = pools
    tsl = slice(tt * TT, (tt + 1) * TT)
    x_t = a1s.tile([P, CH, TT], F32R, name="x_t")
    nc.sync.dma_start(x_t[:], E.xT_r[:, :, tsl])
    e_t = a1s.tile([P, CH, TT], BF16, name="e_t")
    nc.sync.dma_start(e_t[:], E.embT_r[:, :, tsl])

    # LN stats over channels (partition reduction via matmul)
    ps_s = ps_st.tile([1, TT], F32, name="ps_s", tag="ps_st")
    ps_s2 = ps_st.tile([1, TT], F32, name="ps_s2", tag="ps_st")
    for ch in range(CH):
        x2c = a1r.tile([P, TT], F32R, name="x2c", tag="x2c")
        nc.scalar.square(x2c[:], x_t[:, ch, :])
        nc.tensor.matmul(ps_s[:], E.ones_col[:], x_t[:, ch, :],
                         start=(ch == 0), stop=(ch == CH - 1))
        nc.tensor.matmul(ps_s2[:], E.ones_col[:], x2c[:],
                         start=(ch == 0), stop=(ch == CH - 1))
    mu = a1r.tile([1, TT], F32, name="mu")
    m2 = a1r.tile([1, TT], F32, name="m2")
    vr = a1r.tile([1, TT], F32, name="vr")
    nmr = a1r.tile([1, TT], F32, name="nmr")
    nc.vector.tensor_scalar_mul(mu[:], ps_s[:], 1.0 / C)
    nc.vector.tensor_tensor(m2[:], mu[:], mu[:], MUL)
    # var = ps_s2/C - mu^2
    nc.vector.scalar_tensor_tensor(vr[:], ps_s2[:], 1.0 / C, m2[:], MUL, SUB)
    # rstd = exp(-0.5*ln(var+eps)): stays in the exp/ln ACT table set.
    nc.scalar.activation(vr[:], vr[:], LOGN, bias=E.eps_t[0:1], scale=1.0)
    nc.scalar.activation(vr[:], vr[:], EXP, bias=0.0, scale=-0.5)
    rstd = vr
    nc.vector.scalar_tensor_tensor(nmr[:], mu[:], -1.0, rstd[:], MUL, MUL)

    rows_d = dram.tile([2, TT], F32, name="rows_d")
    nc.sync.dma_start(rows_d[0:1, :], rstd[:])
    nc.sync.dma_start(rows_d[1:2, :], nmr[:])
    rstd_bc = a1r.tile([P, TT], F32, name="rstd_bc")
    nc.sync.dma_start(rstd_bc[:], rows_d[0:1, :].to_broadcast((P, TT)))
    nmr_bc = a1r.tile([P, TT], F32, name="nmr_bc")
    nc.sync.dma_start(nmr_bc[:], rows_d[1:2, :].to_broadcast((P, TT)))

    for ch in range(CH):
        ps_sc = ps_a1.tile([P, TT], F32, name="ps_sc", tag="ps_a1")
        for ech in range(CH):
            nc.tensor.matmul(ps_sc[:], E.wemb_sb[:, ech, ch * P:(ch + 1) * P],
                             e_t[:, ech, :],
                             start=(ech == 0), stop=(ech == CH - 1))
        ps_sh = ps_a1.tile([P, TT], F32, name="ps_sh", tag="ps_a1")
        for ech in range(CH):
            nc.tensor.matmul(ps_sh[:], E.wemb_sb[:, ech, C + ch * P:C + (ch + 1) * P],
                             e_t[:, ech, :],
                             start=(ech == 0), stop=(ech == CH - 1))
        # ACT: PSUM->SBUF move + per-partition bias add
        sc_sb = a1r.tile([P, TT], F32, name="sc_sb", tag="sc_sb")
        nc.scalar.add(sc_sb[:], ps_sc[:], E.bemb_sb[:, ch:ch + 1])
        sh_sb = a1r.tile([P, TT], F32, name="sh_sb", tag="sh_sb")
        nc.scalar.add(sh_sb[:], ps_sh[:], E.bemb_sb[:, CH + ch:CH + ch + 1])
        # DVE: xn = x*rstd + (-mu*rstd)
        xn = a1r.tile([P, TT], F32R, name="xn", tag="xn")
        nc.vector.tensor_tensor(xn[:], x_t[:, ch, :], rstd_bc[:], MUL)
        nc.vector.tensor_tensor(xn[:], xn[:], nmr_bc[:], ADD)
        # GPSIMD: h = xn*sc + sh
        tb = a1r.tile([P, TT], F32R, name="tb", tag="tb")
        nc.gpsimd.tensor_tensor(tb[:], xn[:], sc_sb[:], MUL)
        nc.vector.tensor_tensor(E.h_sb[:, ch, tsl], tb[:], sh_sb[:], ADD)


def _phase_a1(E):
    nc, tc = E.nc, E.tc
    with (
        tc.tile_pool(name="wembp", bufs=1) as wembp,
        tc.tile_pool(name="a1s", bufs=2) as a1s,
        tc.tile_pool(name="a1r", bufs=2) as a1r,
        tc.tile_pool(name="ps_a1", bufs=4, space="PSUM") as ps_a1,
        tc.tile_pool(name="ps_st", bufs=2, space="PSUM") as ps_st,
    ):
        E.wemb_sb = wembp.tile([P, CH, O2], BF16, name="wemb_sb")
        nc.sync.dma_start(E.wemb_sb[:], E.WembT_r)
        for tt in range(NTT):
            _a1_tile(E, (a1s, a1r, ps_a1, ps_st), tt)
        if E.debug:
            nc.gpsimd.dma_start(E.dbg_h[:], E.h_sb[:])


def _b_kproj(E, pools, hp):
    """k projection + stats for one head pair; returns (ka0, ka1, rk8)."""
    nc, dram = E.nc, E.dram
    wqk_sb = E.bw.tile([P, CH, 256], F32R, name="wqk_sb")
    nc.sync.dma_start(wqk_sb[:, :, 0:128], E.WprojT_r[:, :, hp * P:(hp + 1) * P])
    nc.sync.dma_start(wqk_sb[:, :, 128:256],
                      E.WprojT_r[:, :, C + hp * P:C + (hp + 1) * P])

    ka0 = E.bka.tile([65, T], BF16, name="ka0")
    ka1 = E.bka.tile([65, T], BF16, name="ka1")
    skexd0 = dram.tile([2, T], F32, name="skexd0")
    skexd1 = dram.tile([2, T], F32, name="skexd1")
    for mt in range(NMT):
        msl = slice(mt * 512, (mt + 1) * 512)
        ps_k = E.ps.tile([P, 512], F32, name="ps_k", tag="ps")
        for ch in range(CH):
            nc.tensor.matmul(ps_k[:], wqk_sb[:, ch, 128:256], E.h_sb[:, ch, msl],
                             start=(ch == 0), stop=(ch == CH - 1))
        k2sb = E.bs0.tile([P, 512], F32R, name="k2sb", tag="k2sb")
        nc.vector.tensor_copy(k2sb[:], ps_k[:])
        ksq = E.bs0.tile([P, 512], F32R, name="ksq", tag="ksq")
        nc.gpsimd.tensor_tensor(ksq[:], k2sb[:], k2sb[:], MUL)
        nc.scalar.copy(ka0[0:64, msl], ps_k[0:64, :])
        nc.scalar.copy(ka1[0:64, msl], ps_k[64:128, :])
        # rows 0/64: 8*mu_k; rows 1/65: E[k^2] (one accumulation group)
        ps_kr = E.ps.tile([P, 512], F32, name="ps_kr", tag="ps")
        nc.tensor.matmul(ps_kr[0:66, :], E.bo8k[:], k2sb[:], start=True, stop=False)
        nc.tensor.matmul(ps_kr[0:66, :], E.bo64k[:], ksq[:], start=False, stop=True)
        skex0 = E.bs0.tile([2, 512], F32, name="skex0", tag="skex0")
        skex1 = E.bs0.tile([2, 512], F32, name="skex1", tag="skex1")
        nc.vector.tensor_copy(skex0[:], ps_kr[0:2, :])
        nc.vector.tensor_copy(skex1[:], ps_kr[64:66, :])
        # k_aug row 64 = 8*mu_k
        nc.vector.tensor_copy(ka0[64:65, msl], skex0[0:1, :])
        nc.vector.tensor_copy(ka1[64:65, msl], skex1[0:1, :])
        nc.sync.dma_start(skexd0[:, msl], skex0[:])
        nc.sync.dma_start(skexd1[:, msl], skex1[:])

    # rk/8 in column form [P, NMC, 2] via DRAM gather
    sk8T = E.bs2.tile([P, NMC, 2], F32, name="sk8T", tag="sk8T")
    ex2kT = E.bs2.tile([P, NMC, 2], F32, name="ex2kT", tag="ex2kT")
    for h_, sd in ((0, skexd0), (1, skexd1)):
        nc.sync.dma_start(sk8T[:, :, h_], sd[0].rearrange("(mc p) -> p mc", p=P))
        nc.sync.dma_start(ex2kT[:, :, h_], sd[1].rearrange("(mc p) -> p mc", p=P))
    rk8 = E.bs2.tile([P, NMC, 2], F32, name="rk8", tag="rk8")
    nc.vector.tensor_tensor(rk8[:], sk8T[:], sk8T[:], MUL)  # 64*mu^2
    # var = E[k^2] - (8mu)^2/64
    nc.vector.scalar_tensor_tensor(rk8[:], rk8[:], -1.0 / 64, ex2kT[:], MUL, ADD)
    # rk/8 = exp(-0.5*ln(var+eps) + ln(1/8))
    nc.scalar.activation(rk8[:], rk8[:], LOGN, bias=E.eps_t[:], scale=1.0)
    nc.scalar.activation(rk8[:], rk8[:], EXP, bias=E.ln8_t[:], scale=-0.5)
    return wqk_sb, ka0, ka1, rk8


def _b_qproj(E, wqk_sb):
    """q projection + stats for one head pair; returns (qa0, qa1)."""
    nc, dram = E.nc, E.dram
    q2 = E.bq.tile([P, TQ], BF16, name="q2")
    nsq8 = E.bs1.tile([65, TQ], F32, name="nsq8", tag="nsq8")
    ex2q = E.bs1.tile([65, TQ], F32, name="ex2q", tag="ex2q")
    for nt in range(TQ // 512):
        nsl = slice(nt * 512, (nt + 1) * 512)
        ps_q = E.ps.tile([P, 512], F32, name="ps_q", tag="ps")
        for ch in range(CH):
            nc.tensor.matmul(ps_q[:], wqk_sb[:, ch, 0:128], E.h_sb[:, ch, nsl],
                             start=(ch == 0), stop=(ch == CH - 1))
        nc.vector.tensor_copy(q2[:, nsl], ps_q[:])
        qsq = E.bs0.tile([P, 512], F32R, name="qsq", tag="qsq")
        nc.gpsimd.tensor_tensor(qsq[:], q2[:, nsl], q2[:, nsl], MUL)
        ps_qr = E.ps.tile([P, 512], F32, name="ps_qr", tag="ps")
        nc.tensor.matmul(ps_qr[0:65, :], E.bon8[:], q2[:, nsl], start=True, stop=True)
        ps_qr2 = E.ps.tile([P, 512], F32, name="ps_qr2", tag="ps")
        nc.tensor.matmul(ps_qr2[0:65, :], E.bo64q[:], qsq[:], start=True, stop=True)
        nc.vector.tensor_copy(nsq8[:, nsl], ps_qr[0:65, :])
        nc.vector.tensor_copy(ex2q[:, nsl], ps_qr2[0:65, :])

    rq = E.bs1.tile([65, TQ], F32, name="rq", tag="rq")
    nc.gpsimd.tensor_tensor(rq[:], nsq8[:], nsq8[:], MUL)
    nc.vector.scalar_tensor_tensor(rq[:], rq[:], -1.0 / 64, ex2q[:], MUL, ADD)
    nc.scalar.activation(rq[:], rq[:], LOGN, bias=E.eps_t[0:65], scale=1.0)
    nc.scalar.activation(rq[:], rq[:], EXP, bias=0.0, scale=-0.5)
    rq_d = dram.tile([2, TQ], F32, name="rq_d")
    nc.sync.dma_start(rq_d[0:1, :], rq[0:1, :])
    nc.sync.dma_start(rq_d[1:2, :], rq[64:65, :])
    rq_bc = E.bs1.tile([P, TQ], F32, name="rq_bc", tag="rq_bc")
    nc.sync.dma_start(rq_bc[0:64, :], rq_d[0:1, :].to_broadcast((64, TQ)))
    nc.sync.dma_start(rq_bc[64:128, :], rq_d[1:2, :].to_broadcast((64, TQ)))

    qa0 = E.bq.tile([65, TQ], BF16, name="qa0")
    qa1 = E.bq.tile([65, TQ], BF16, name="qa1")
    nc.vector.tensor_tensor(qa0[0:64, :], q2[0:64, :], rq_bc[0:64, :], MUL)
    nc.vector.tensor_tensor(qa1[0:64, :], q2[64:128, :], rq_bc[64:128, :], MUL)
    nc.vector.tensor_tensor(qa0[64:65, :], nsq8[0:1, :], rq[0:1, :], MUL)
    nc.vector.tensor_tensor(qa1[64:65, :], nsq8[64:65, :], rq[64:65, :], MUL)
    return qa0, qa1


def _b_scores(E, v_sb, hp, hh, ka, qa, rk8):
    """softmax(qk^T)v for one head over all TQ queries."""
    nc, dram = E.nc, E.dram
    head = 2 * hp + hh
    vidx = (hp % 2) * 2 + hh
    ps_ov = E.ps_o.tile([65, TQ], F32, name="ps_ov", tag="ps_o")
    for mc in range(NMC):
        # matmul output is limited to one PSUM bank (N<=512): run the two
        # query halves as separate matmuls into the 2-bank tile, then one
        # [128,1024] exp over both.
        ps_sT = E.ps_big.tile([P, TQ], F32, name="ps_sT", tag="ps_big")
        for half in range(2):
            hsl = slice(half * 512, (half + 1) * 512)
            nc.tensor.matmul(ps_sT[:, hsl], ka[:, mc * P:(mc + 1) * P],
                             qa[:, hsl], start=True, stop=True)
        p_t = E.bp.tile([P, TQ], BF16, name="p_t", tag="p_t")
        nc.scalar.activation(p_t[:], ps_sT[:], EXP,
                             bias=0.0, scale=rk8[:, mc, hh:hh + 1])
        if E.debug and head == 0 and mc < 2:
            nc.sync.dma_start(E.dbg_p[:, mc * TQ:(mc + 1) * TQ], p_t[:])
        for half in range(2):
            hsl = slice(half * 512, (half + 1) * 512)
            nc.tensor.matmul(ps_ov[:, hsl], v_sb[:, mc, vidx, 0:65], p_t[:, hsl],
                             start=(mc == 0), stop=(mc == NMC - 1))
    # drain: unnormalized o + denominator reciprocal
    o_u = E.bo.tile([64, TQ], BF16, name="o_u")
    nc.scalar.copy(o_u[:], ps_ov[0:64, :])
    den = E.bo1.tile([1, TQ], F32, name="den")
    nc.vector.tensor_copy(den[:], ps_ov[64:65, :])
    rec = E.bo1.tile([1, TQ], F32, name="rec")
    nc.vector.reciprocal_approx_fast(out=rec[:], in_=den[:])
    rec_d = dram.tile([1, TQ], F32, name="rec_d")
    nc.sync.dma_start(rec_d[:], rec[:])
    if E.debug and head == 0:
        nc.sync.dma_start(E.dbg_ou[:], o_u[:])
        nc.sync.dma_start(E.dbg_rec[:], rec[:])
    rec_bc = E.bo1.tile([64, TQ], F32, name="rec_bc")
    nc.sync.dma_start(rec_bc[:], rec_d[:].to_broadcast((64, TQ)))
    nc.vector.tensor_tensor(
        E.o_fm[(head % 2) * 64:(head % 2) * 64 + 64, head // 2, :],
        o_u[:], rec_bc[:], MUL)


def _phase_b(E):
    nc, tc = E.nc, E.tc
    with (
        tc.tile_pool(name="bwv", bufs=1) as bwv,
        tc.tile_pool(name="bw", bufs=2) as bw,
        tc.tile_pool(name="bv", bufs=2) as bv,
        tc.tile_pool(name="bka", bufs=2) as bka,
        tc.tile_pool(name="bq", bufs=2) as bq,
        tc.tile_pool(name="bs1", bufs=1) as bs1,
        tc.tile_pool(name="bs0", bufs=1) as bs0,
        tc.tile_pool(name="bs2", bufs=2) as bs2,
        tc.tile_pool(name="bp", bufs=2) as bp,
        tc.tile_pool(name="bo", bufs=2) as bo,
        tc.tile_pool(name="bo1", bufs=1) as bo1,
        tc.tile_pool(name="ps", bufs=2, space="PSUM") as ps,
        tc.tile_pool(name="ps_big", bufs=2, space="PSUM") as ps_big,
        tc.tile_pool(name="ps_o", bufs=1, space="PSUM") as ps_o,
    ):
        E.bw, E.bka, E.bq = bw, bka, bq
        E.bs0, E.bs1, E.bs2 = bs0, bs1, bs2
        E.bp, E.bo, E.bo1 = bp, bo, bo1
        E.ps, E.ps_big, E.ps_o = ps, ps_big, ps_o
        for hq in range(4):  # head quads
            wv_sb = bwv.tile([P, CH, 256], F32R, name="wv_sb")
            nc.sync.dma_start(wv_sb[:], E.WprojT_r[:, :, 2 * C + hq * 256:2 * C + (hq + 1) * 256])
            v_sb = bv.tile([P, NMC, 4, 72], BF16, name="v_sb")
            nc.vector.memset(v_sb[:, :, :, 64:65], 1.0)
            for mtk in range(NMC):
                ps_v = ps.tile([P, 512], F32, name="ps_v", tag="ps")
                for ch in range(CH):
                    nc.tensor.matmul(ps_v[:, 0:256], E.h_sb[:, ch, mtk * P:(mtk + 1) * P],
                                     wv_sb[:, ch, :], start=(ch == 0), stop=(ch == CH - 1))
                # one strided copy: [128, 4 heads, 64], head stride 72 in dest
                nc.vector.tensor_copy(v_sb[:, mtk, :, 0:64],
                                      ps_v[:, 0:256].rearrange("p (h d) -> p h d", h=4))

            hp_data = []
            for hp in (2 * hq, 2 * hq + 1):
                wqk_sb, ka0, ka1, rk8 = _b_kproj(E, None, hp)
                qa0, qa1 = _b_qproj(E, wqk_sb)
                if E.debug and hp == 0:
                    nc.sync.dma_start(E.dbg_ka0[:], ka0[:])
                    nc.sync.dma_start(E.dbg_ka1[:], ka1[:])
                    nc.sync.dma_start(E.dbg_rk8[:], rk8[:].rearrange("p a b -> p (a b)"))
                    nc.sync.dma_start(E.dbg_qa0[:], qa0[:])
                    nc.sync.dma_start(E.dbg_qa1[:], qa1[:])
                hp_data.append((hp, (ka0, qa0), (ka1, qa1), rk8))

            for hp, h0, h1, rk8 in hp_data:
                for hh, (ka, qa) in enumerate((h0, h1)):
                    _b_scores(E, v_sb, hp, hh, ka, qa, rk8)


def _phase_c(E):
    nc, tc = E.nc, E.tc
    with (
        tc.tile_pool(name="cw2", bufs=1) as cw2,
        tc.tile_pool(name="ps_c", bufs=2, space="PSUM") as ps_c,
    ):
        for jt in range(C // 512):
            if jt == 0:
                wres_sb = E.wres0
            else:
                wres_sb = cw2.tile([P, CH, 512], BF16, name="wres_sb")
                nc.sync.dma_start(wres_sb[:], E.WresT_r[:, :, jt * 512:(jt + 1) * 512])
            for ns in range(TQ // P):
                ps_f = ps_c.tile([P, 512], F32, name="ps_f", tag="ps_c")
                for cg in range(CH):
                    nc.tensor.matmul(ps_f[:], E.o_fm[:, cg, ns * P:(ns + 1) * P],
                                     wres_sb[:, cg, :],
                                     start=(cg == 0), stop=(cg == CH - 1))
                f_sb = cw2.tile([P, 512], F32, name="f_sb")
                nc.scalar.copy(f_sb[:], ps_f[:])
                nc.sync.dma_start(E.out[ns * P:(ns + 1) * P, jt * 512:(jt + 1) * 512],
                                  f_sb[:])


def build_kernel(debug=False):
    import concourse.bacc as _bacc_mod
    _orig_tables = _bacc_mod.get_activation_tables

    def _one_set(arch):
        return {k: (v if k == "natural_log_exp_and_others" else frozenset())
                for k, v in _orig_tables(arch).items()}

    _bacc_mod.get_activation_tables = _one_set
    try:
        return _build_kernel_inner(debug)
    finally:
        _bacc_mod.get_activation_tables = _orig_tables


def _build_kernel_inner(debug=False):
    nc = bacc.Bacc()
    E = SimpleNamespace(nc=nc, debug=debug)

    E.xT = nc.dram_tensor("xT", [C, T], F32R, kind="ExternalInput")
    E.embT = nc.dram_tensor("embT", [EMB, T], BF16, kind="ExternalInput")
    E.WembT = nc.dram_tensor("WembT", [EMB, O2], BF16, kind="ExternalInput")
    E.bemb = nc.dram_tensor("bemb", [P, O2 // P], F32, kind="ExternalInput")
    E.WprojT = nc.dram_tensor("WprojT", [C, 3 * C], F32R, kind="ExternalInput")
    E.WresT = nc.dram_tensor("WresT", [C, C], BF16, kind="ExternalInput")
    E.out = nc.dram_tensor("out", [TQ, C], F32, kind="ExternalOutput")
    if debug:
        E.dbg_h = nc.dram_tensor("dbg_h", [P, CH, T], F32, kind="ExternalOutput")
        E.dbg_ka0 = nc.dram_tensor("dbg_ka0", [65, T], BF16, kind="ExternalOutput")
        E.dbg_ka1 = nc.dram_tensor("dbg_ka1", [65, T], BF16, kind="ExternalOutput")
        E.dbg_rk8 = nc.dram_tensor("dbg_rk8", [P, NMC * 2], F32, kind="ExternalOutput")
        E.dbg_qa0 = nc.dram_tensor("dbg_qa0", [65, TQ], BF16, kind="ExternalOutput")
        E.dbg_qa1 = nc.dram_tensor("dbg_qa1", [65, TQ], BF16, kind="ExternalOutput")
        E.dbg_p = nc.dram_tensor("dbg_p", [P, 2 * TQ], BF16, kind="ExternalOutput")
        E.dbg_ou = nc.dram_tensor("dbg_ou", [64, TQ], BF16, kind="ExternalOutput")
        E.dbg_rec = nc.dram_tensor("dbg_rec", [1, TQ], F32, kind="ExternalOutput")

    E.xT_r = E.xT.rearrange("(ch p) t -> p ch t", p=P)
    E.embT_r = E.embT.rearrange("(ch p) t -> p ch t", p=P)
    E.WembT_r = E.WembT.rearrange("(ch p) o -> p ch o", p=P)
    E.WprojT_r = E.WprojT.rearrange("(ch p) o -> p ch o", p=P)
    E.WresT_r = E.WresT.rearrange("(ch p) o -> p ch o", p=P)

    with tile.TileContext(nc) as tc:
        E.tc = tc
        with (
            tc.tile_pool(name="const", bufs=1) as const,
            tc.tile_pool(name="dram", bufs=2, space="DRAM") as dram,
        ):
            E.const, E.dram = const, dram
            _consts(E)
            with tc.tile_pool(name="hpool", bufs=1) as hpool:
                E.h_sb = hpool.tile([P, CH, T], F32R, name="h_sb")   # 64KB/part
                E.o_fm = hpool.tile([P, CH, TQ], BF16, name="o_fm")  # 16KB/part
                _phase_a1(E)
                with tc.tile_pool(name="cw", bufs=1) as cw:
                    E.cw = cw
                    E.wres0 = cw.tile([P, CH, 512], BF16, name="wres0")
                    nc.sync.dma_start(E.wres0[:], E.WresT_r[:, :, 0:512])
                    _phase_b(E)
                    _phase_c(E)

    nc.finalize()
    return nc


def _prep_host(x, emb, W_emb, b_emb, W_proj, W_out):
    BF = ml_dtypes.bfloat16
    W_embT = np.ascontiguousarray(W_emb.T).astype(BF)
    W_projT = np.ascontiguousarray(W_proj.T.astype(np.float32))
    W_resT = np.ascontiguousarray((np.eye(C, dtype=np.float32) + W_out).T).astype(BF)
    bemb2 = b_emb.astype(np.float32).copy()
    bemb2[:C] += 1.0                       # fold the FiLM "+1" into the bias
    bemb_col = np.ascontiguousarray(bemb2.reshape(O2 // P, P).T)

    in_maps = []
    for c in range(NCORES):
        b, j = c // 2, c % 2
        perm = np.concatenate([np.arange(j * TQ, (j + 1) * TQ),
                               np.arange((1 - j) * TQ, (2 - j) * TQ)])
        in_maps.append({
            "xT": np.ascontiguousarray(x[b][perm].T.astype(np.float32)),
            "embT": np.ascontiguousarray(emb[b][perm].T).astype(BF),
            "WembT": W_embT, "bemb": bemb_col,
            "WprojT": W_projT, "WresT": W_resT,
        })
    return in_maps


def kernel(x, emb, W_emb, b_emb, W_proj, W_out, _trace=False, _debug=False):
    x = np.asarray(x); emb = np.asarray(emb)
    W_emb = np.asarray(W_emb); b_emb = np.asarray(b_emb)
    W_proj = np.asarray(W_proj); W_out = np.asarray(W_out)

    key = "nc_dbg" if _debug else "nc"
    if key not in _cached:
        _cached[key] = build_kernel(debug=_debug)
    nc = _cached[key]

    in_maps = _prep_host(x, emb, W_emb, b_emb, W_proj, W_out)
    res = run_bass_kernel_spmd(nc, in_maps, core_ids=list(range(NCORES)), trace=_trace)
    _cached["last_result"] = res

    outp = np.empty((B, N, C), dtype=np.float32)
    for c in range(NCORES):
        b, j = c // 2, c % 2
        outp[b, j * TQ:(j + 1) * TQ, :] = res.results[c]["out"]
    _cached["last_out"] = outp
    return outp


# revision 16
# speedup vs baseline: 1.0581x; 1.0581x over previous
"""AttentionBlock Trainium2 kernel (B=4, N=2048, C=1024, H=16, D=64, EMB=1024).

    se = emb @ W_emb.T + b_emb;  scale, shift = split(se, 2, -1)
    h  = LN(x) * (1+scale) + shift
    q,k,v = split(h @ W_proj.T) -> (B,H,N,D);  q = LN(q); k = LN(k)  (over D)
    o  = softmax(q k^T / sqrt(D)) v  -> (B,N,C)
    out = o + o @ W_out.T

Sharding: 8 cores; core c -> batch b=c//2, query-half j=c%2. The host rolls
the token axis per core so its query tokens are always tokens 0:1024
(attention is permutation-equivariant over key/value tokens), giving one
symmetric SPMD NEFF with no collectives. Each core computes the full-batch
preamble (se/h/k/v over all 2048 tokens), and q/attention/out-proj for its
1024 rows.

Dataflow is feature-major (channels on partitions) end to end:
  - LayerNorm over channels == partition reduction -> ones-column matmuls.
  - Per-token (free-dim) scalars broadcast across partitions by bouncing a
    row through DRAM (DRAM APs allow step-0 partition dims; SBUF APs don't).
  - q/k LN centering rides the score matmul as an augmented 65th row
    (k_aug row64 = 8*mu_k[m], q_aug row64 = -8*mu_q[n]*rq[n]); the rk[m]/8
    factor is applied by the ACT exp per-partition scale operand.
  - k-side stat row-matmuls share one accumulation group / PSUM tile:
    cols 0/64 of the 1/8-selector give 8*mu at rows 0/64, cols 1/65 of the
    1/64-selector give E[k^2] at rows 1/65.
  - Softmax denominators come free as a ones column appended to v; the
    division is deferred until after the attn@v matmul.
  - The residual is folded into the output projection: W_res = (I+W_out).T.

Perf structure (second generation of this kernel):
  - Scores run per (head, key-chunk) over the FULL 1024 query tokens so the
    exp consumes a 2-bank [128,1024] PSUM tile in one ACT instruction
    (amortizes ACT's ~440-cycle SBUF access latency), and the attn@v matmul
    is a single N=1024 bf16 instruction per key chunk.
  - rsqrt = exp(-0.5*ln(var+eps)) on ACT keeps every activation used (Exp,
    Ln, Copy/Identity) inside the one `natural_log_exp_and_others` table
    set: no ACT table reloads.
  - Softmax reciprocal via the 1-instruction DVE approx (~18 bits); the
    unnormalized o and the reciprocal row are staged out of PSUM so the
    accumulator bank frees immediately.
  - GPSIMD (no PSUM port) takes SBUF->SBUF f32 work: squares for the
    variance stats and the FiLM multiply-add tail. ACT takes per-partition
    bias adds and one of the two k PSUM->SBUF head copies.
  - bf16 for: emb/W_emb (se matmul), q/k score operands, attention
    probabilities and v, o_fm and W_res. f32r elsewhere.
  - Head-quad software pipelining: both head-pairs of a quad are projected
    before either one's score phase, so the q/k-stat latency chains hide
    under projection matmuls and the PE never idles long enough for the
    HAM clock gate to re-throttle.
"""

import sys

sys.path.insert(0, "/opt/trn_rl_repo")

import math
from types import SimpleNamespace

import ml_dtypes
import numpy as np

import concourse.bass as bass
import concourse.mybir as mybir
import concourse.tile as tile
from concourse import bacc
from concourse.bass_utils import run_bass_kernel_spmd

P = 128
B, N, C = 4, 2048, 1024
H, D = 16, 64
EMB = 1024
EPS = 1e-5
T = N          # tokens per batch on each core (k/v coverage)
TQ = N // 2    # query tokens per core
CH = C // P    # 8 channel chunks
O2 = 2 * C
NCORES = 8
TT = 512       # token tile in phase A1
NTT = T // TT  # 4
NMT = T // 512   # 4  key-token tiles (512)
NMC = T // P     # 16 key-token chunks (128)
LN8 = math.log(0.125)

F32 = mybir.dt.float32
F32R = mybir.dt.float32r
BF16 = mybir.dt.bfloat16
MUL = mybir.AluOpType.mult
ADD = mybir.AluOpType.add
SUB = mybir.AluOpType.subtract
EXP = mybir.ActivationFunctionType.Exp
LOGN = mybir.ActivationFunctionType.Ln

_cached = {}


def _consts(E):
    nc, const = E.nc, E.const
    E.eps_t = const.tile([P, 1], F32, name="eps_t")
    nc.vector.memset(E.eps_t[:], EPS)
    E.ln8_t = const.tile([P, 1], F32, name="ln8_t")
    nc.vector.memset(E.ln8_t[:], LN8)
    # memset can't emit float32r: stage constants in F32, copy-round.
    cscr = const.tile([P, 66], F32, name="cscr")
    E.ones_col = const.tile([P, 1], F32R, name="ones_col")
    nc.vector.memset(cscr[:, 0:1], 1.0)
    nc.vector.tensor_copy(E.ones_col[:], cscr[:, 0:1])
    # k row stats: one group, two matmuls. bo8k puts 8*mu at rows 0/64
    # (cols 0/64), bo64k puts E[k^2] at rows 1/65 (cols 1/65).
    E.bo8k = const.tile([P, 66], F32R, name="bo8k")
    E.bo64k = const.tile([P, 66], F32R, name="bo64k")
    # q row stats: separate tiles, rows 0/64 in each.
    E.bon8 = const.tile([P, 65], F32R, name="bon8")    # -1/8
    E.bo64q = const.tile([P, 65], F32R, name="bo64q")  # +1/64
    for t_, v_, c0, c1, w in ((E.bo8k, 0.125, 0, 64, 66),
                              (E.bo64k, 1.0 / 64, 1, 65, 66),
                              (E.bon8, -0.125, 0, 64, 65),
                              (E.bo64q, 1.0 / 64, 0, 64, 65)):
        nc.vector.memset(cscr[:], 0.0)
        nc.vector.memset(cscr[0:64, c0:c0 + 1], v_)
        nc.vector.memset(cscr[64:128, c1:c1 + 1], v_)
        nc.vector.tensor_copy(t_[:], cscr[:, 0:w])
    E.bemb_sb = const.tile([P, O2 // P], F32, name="bemb_sb")
    nc.sync.dma_start(E.bemb_sb[:], E.bemb[:])


def _a1_tile(E, pools, tt):
    """One 512-token tile of phase A1: stats, se matmuls, FiLM."""
    nc, dram = E.nc, E.dram
    a1s, a1r, ps_a1, ps_st = pools
    tsl = slice(tt * TT, (tt + 1) * TT)
    x_t = a1s.tile([P, CH, TT], F32R, name="x_t")
    nc.sync.dma_start(x_t[:], E.xT_r[:, :, tsl])
    e_t = a1s.tile([P, CH, TT], BF16, name="e_t")
    nc.sync.dma_start(e_t[:], E.embT_r[:, :, tsl])

    # LN stats over channels (partition reduction via matmul)
    ps_s = ps_st.tile([1, TT], F32, name="ps_s", tag="ps_st")
    ps_s2 = ps_st.tile([1, TT], F32, name="ps_s2", tag="ps_st")
    for ch in range(CH):
        x2c = a1r.tile([P, TT], F32R, name="x2c", tag="x2c")
        nc.scalar.square(x2c[:], x_t[:, ch, :])
        nc.tensor.matmul(ps_s[:], E.ones_col[:], x_t[:, ch, :],
                         start=(ch == 0), stop=(ch == CH - 1))
        nc.tensor.matmul(ps_s2[:], E.ones_col[:], x2c[:],
                         start=(ch == 0), stop=(ch == CH - 1))
    mu = a1r.tile([1, TT], F32, name="mu")
    m2 = a1r.tile([1, TT], F32, name="m2")
    vr = a1r.tile([1, TT], F32, name="vr")
    nmr = a1r.tile([1, TT], F32, name="nmr")
    nc.vector.tensor_scalar_mul(mu[:], ps_s[:], 1.0 / C)
    nc.vector.tensor_tensor(m2[:], mu[:], mu[:], MUL)
    # var = ps_s2/C - mu^2
    nc.vector.scalar_tensor_tensor(vr[:], ps_s2[:], 1.0 / C, m2[:], MUL, SUB)
    # rstd = exp(-0.5*ln(var+eps)): stays in the exp/ln ACT table set.
    nc.scalar.activation(vr[:], vr[:], LOGN, bias=E.eps_t[0:1], scale=1.0)
    nc.scalar.activation(vr[:], vr[:], EXP, bias=0.0, scale=-0.5)
    rstd = vr
    nc.vector.scalar_tensor_tensor(nmr[:], mu[:], -1.0, rstd[:], MUL, MUL)

    rows_d = dram.tile([2, TT], F32, name="rows_d")
    nc.sync.dma_start(rows_d[0:1, :], rstd[:])
    nc.sync.dma_start(rows_d[1:2, :], nmr[:])
    rstd_bc = a1r.tile([P, TT], F32, name="rstd_bc")
    nc.sync.dma_start(rstd_bc[:], rows_d[0:1, :].to_broadcast((P, TT)))
    nmr_bc = a1r.tile([P, TT], F32, name="nmr_bc")
    nc.sync.dma_start(nmr_bc[:], rows_d[1:2, :].to_broadcast((P, TT)))

    for ch in range(CH):
        ps_sc = ps_a1.tile([P, TT], F32, name="ps_sc", tag="ps_a1")
        for ech in range(CH):
            nc.tensor.matmul(ps_sc[:], E.wemb_sb[:, ech, ch * P:(ch + 1) * P],
                             e_t[:, ech, :],
                             start=(ech == 0), stop=(ech == CH - 1))
        ps_sh = ps_a1.tile([P, TT], F32, name="ps_sh", tag="ps_a1")
        for ech in range(CH):
            nc.tensor.matmul(ps_sh[:], E.wemb_sb[:, ech, C + ch * P:C + (ch + 1) * P],
                             e_t[:, ech, :],
                             start=(ech == 0), stop=(ech == CH - 1))
        # ACT: PSUM->SBUF move + per-partition bias add
        sc_sb = a1r.tile([P, TT], F32, name="sc_sb", tag="sc_sb")
        nc.scalar.add(sc_sb[:], ps_sc[:], E.bemb_sb[:, ch:ch + 1])
        sh_sb = a1r.tile([P, TT], F32, name="sh_sb", tag="sh_sb")
        nc.scalar.add(sh_sb[:], ps_sh[:], E.bemb_sb[:, CH + ch:CH + ch + 1])
        # DVE: xn = x*rstd + (-mu*rstd)
        xn = a1r.tile([P, TT], F32R, name="xn", tag="xn")
        nc.vector.tensor_tensor(xn[:], x_t[:, ch, :], rstd_bc[:], MUL)
        nc.vector.tensor_tensor(xn[:], xn[:], nmr_bc[:], ADD)
        # GPSIMD: h = xn*sc + sh
        tb = a1r.tile([P, TT], F32R, name="tb", tag="tb")
        nc.gpsimd.tensor_tensor(tb[:], xn[:], sc_sb[:], MUL)
        nc.vector.tensor_tensor(E.h_sb[:, ch, tsl], tb[:], sh_sb[:], ADD)


def _phase_a1(E):
    nc, tc = E.nc, E.tc
    with (
        tc.tile_pool(name="wembp", bufs=1) as wembp,
        tc.tile_pool(name="a1s", bufs=2) as a1s,
        tc.tile_pool(name="a1r", bufs=2) as a1r,
        tc.tile_pool(name="ps_a1", bufs=4, space="PSUM") as ps_a1,
        tc.tile_pool(name="ps_st", bufs=2, space="PSUM") as ps_st,
    ):
        E.wemb_sb = wembp.tile([P, CH, O2], BF16, name="wemb_sb")
        nc.sync.dma_start(E.wemb_sb[:], E.WembT_r)
        for tt in range(NTT):
            _a1_tile(E, (a1s, a1r, ps_a1, ps_st), tt)
        if E.debug:
            nc.gpsimd.dma_start(E.dbg_h[:], E.h_sb[:])


def _b_kproj(E, pools, hp):
    """k projection + stats for one head pair; returns (ka0, ka1, rk8)."""
    nc, dram = E.nc, E.dram
    wqk_sb = E.bw.tile([P, CH, 256], F32R, name="wqk_sb")
    nc.sync.dma_start(wqk_sb[:, :, 0:128], E.WprojT_r[:, :, hp * P:(hp + 1) * P])
    nc.sync.dma_start(wqk_sb[:, :, 128:256],
                      E.WprojT_r[:, :, C + hp * P:C + (hp + 1) * P])

    ka0 = E.bka.tile([65, T], BF16, name="ka0")
    ka1 = E.bka.tile([65, T], BF16, name="ka1")
    skexd0 = dram.tile([2, T], F32, name="skexd0")
    skexd1 = dram.tile([2, T], F32, name="skexd1")
    for mt in range(NMT):
        msl = slice(mt * 512, (mt + 1) * 512)
        ps_k = E.ps.tile([P, 512], F32, name="ps_k", tag="ps")
        for ch in range(CH):
            nc.tensor.matmul(ps_k[:], wqk_sb[:, ch, 128:256], E.h_sb[:, ch, msl],
                             start=(ch == 0), stop=(ch == CH - 1))
        k2sb = E.bs0.tile([P, 512], F32R, name="k2sb", tag="k2sb")
        nc.vector.tensor_copy(k2sb[:], ps_k[:])
        ksq = E.bs0.tile([P, 512], F32R, name="ksq", tag="ksq")
        nc.gpsimd.tensor_tensor(ksq[:], k2sb[:], k2sb[:], MUL)
        nc.scalar.copy(ka0[0:64, msl], ps_k[0:64, :])
        nc.scalar.copy(ka1[0:64, msl], ps_k[64:128, :])
        # rows 0/64: 8*mu_k; rows 1/65: E[k^2] (one accumulation group)
        ps_kr = E.ps.tile([P, 512], F32, name="ps_kr", tag="ps")
        nc.tensor.matmul(ps_kr[0:66, :], E.bo8k[:], k2sb[:], start=True, stop=False)
        nc.tensor.matmul(ps_kr[0:66, :], E.bo64k[:], ksq[:], start=False, stop=True)
        skex0 = E.bs0.tile([2, 512], F32, name="skex0", tag="skex0")
        skex1 = E.bs0.tile([2, 512], F32, name="skex1", tag="skex1")
        nc.vector.tensor_copy(skex0[:], ps_kr[0:2, :])
        nc.vector.tensor_copy(skex1[:], ps_kr[64:66, :])
        # k_aug row 64 = 8*mu_k
        nc.vector.tensor_copy(ka0[64:65, msl], skex0[0:1, :])
        nc.vector.tensor_copy(ka1[64:65, msl], skex1[0:1, :])
        nc.sync.dma_start(skexd0[:, msl], skex0[:])
        nc.sync.dma_start(skexd1[:, msl], skex1[:])

    # rk/8 in column form [P, NMC, 2] via DRAM gather
    sk8T = E.bs2.tile([P, NMC, 2], F32, name="sk8T", tag="sk8T")
    ex2kT = E.bs2.tile([P, NMC, 2], F32, name="ex2kT", tag="ex2kT")
    for h_, sd in ((0, skexd0), (1, skexd1)):
        nc.sync.dma_start(sk8T[:, :, h_], sd[0].rearrange("(mc p) -> p mc", p=P))
        nc.sync.dma_start(ex2kT[:, :, h_], sd[1].rearrange("(mc p) -> p mc", p=P))
    rk8 = E.bs2.tile([P, NMC, 2], F32, name="rk8", tag="rk8")
    nc.vector.tensor_tensor(rk8[:], sk8T[:], sk8T[:], MUL)  # 64*mu^2
    # var = E[k^2] - (8mu)^2/64
    nc.vector.scalar_tensor_tensor(rk8[:], rk8[:], -1.0 / 64, ex2kT[:], MUL, ADD)
    # rk/8 = exp(-0.5*ln(var+eps) + ln(1/8))
    nc.scalar.activation(rk8[:], rk8[:], LOGN, bias=E.eps_t[:], scale=1.0)
    nc.scalar.activation(rk8[:], rk8[:], EXP, bias=E.ln8_t[:], scale=-0.5)
    return wqk_sb, ka0, ka1, rk8


def _b_qproj(E, wqk_sb):
    """q projection + stats for one head pair; returns (qa0, qa1)."""
    nc, dram = E.nc, E.dram
    q2 = E.bq.tile([P, TQ], F32R, name="q2")
    nsq8 = E.bs1.tile([65, TQ], F32, name="nsq8", tag="nsq8")
    ex2q = E.bs1.tile([65, TQ], F32, name="ex2q", tag="ex2q")
    for nt in range(TQ // 512):
        nsl = slice(nt * 512, (nt + 1) * 512)
        ps_q = E.ps.tile([P, 512], F32, name="ps_q", tag="ps")
        for ch in range(CH):
            nc.tensor.matmul(ps_q[:], wqk_sb[:, ch, 0:128], E.h_sb[:, ch, nsl],
                             start=(ch == 0), stop=(ch == CH - 1))
        nc.vector.tensor_copy(q2[:, nsl], ps_q[:])
        qsq = E.bs0.tile([P, 512], F32R, name="qsq", tag="qsq")
        nc.gpsimd.tensor_tensor(qsq[:], q2[:, nsl], q2[:, nsl], MUL)
        ps_qr = E.ps.tile([P, 512], F32, name="ps_qr", tag="ps")
        nc.tensor.matmul(ps_qr[0:65, :], E.bon8[:], q2[:, nsl], start=True, stop=True)
        ps_qr2 = E.ps.tile([P, 512], F32, name="ps_qr2", tag="ps")
        nc.tensor.matmul(ps_qr2[0:65, :], E.bo64q[:], qsq[:], start=True, stop=True)
        nc.vector.tensor_copy(nsq8[:, nsl], ps_qr[0:65, :])
        nc.vector.tensor_copy(ex2q[:, nsl], ps_qr2[0:65, :])

    rq = E.bs1.tile([65, TQ], F32, name="rq", tag="rq")
    nc.gpsimd.tensor_tensor(rq[:], nsq8[:], nsq8[:], MUL)
    nc.vector.scalar_tensor_tensor(rq[:], rq[:], -1.0 / 64, ex2q[:], MUL, ADD)
    nc.scalar.activation(rq[:], rq[:], LOGN, bias=E.eps_t[0:65], scale=1.0)
    nc.scalar.activation(rq[:], rq[:], EXP, bias=0.0, scale=-0.5)
    rq_d = dram.tile([2, TQ], F32, name="rq_d")
    nc.sync.dma_start(rq_d[0:1, :], rq[0:1, :])
    nc.sync.dma_start(rq_d[1:2, :], rq[64:65, :])
    rq_bc = E.bs1.tile([P, TQ], F32, name="rq_bc", tag="rq_bc")
    nc.sync.dma_start(rq_bc[0:64, :], rq_d[0:1, :].to_broadcast((64, TQ)))
    nc.sync.dma_start(rq_bc[64:128, :], rq_d[1:2, :].to_broadcast((64, TQ)))

    qa0 = E.bq.tile([65, TQ], BF16, name="qa0")
    qa1 = E.bq.tile([65, TQ], BF16, name="qa1")
    nc.vector.tensor_tensor(qa0[0:64, :], q2[0:64, :], rq_bc[0:64, :], MUL)
    nc.vector.tensor_tensor(qa1[0:64, :], q2[64:128, :], rq_bc[64:128, :], MUL)
    nc.vector.tensor_tensor(qa0[64:65, :], nsq8[0:1, :], rq[0:1, :], MUL)
    nc.vector.tensor_tensor(qa1[64:65, :], nsq8[64:65, :], rq[64:65, :], MUL)
    return qa0, qa1


def _b_scores(E, v_sb, hp, hh, ka, qa, rk8):
    """softmax(qk^T)v for one head over all TQ queries."""
    nc, dram = E.nc, E.dram
    head = 2 * hp + hh
    vidx = (hp % 2) * 2 + hh
    ps_ov = E.ps_o.tile([65, TQ], F32, name="ps_ov", tag="ps_o")
    for mc in range(NMC):
        # matmul output is limited to one PSUM bank (N<=512): run the two
        # query halves as separate matmuls into the 2-bank tile, then one
        # [128,1024] exp over both.
        ps_sT = E.ps_big.tile([P, TQ], F32, name="ps_sT", tag="ps_big")
        for half in range(2):
            hsl = slice(half * 512, (half + 1) * 512)
            nc.tensor.matmul(ps_sT[:, hsl], ka[:, mc * P:(mc + 1) * P],
                             qa[:, hsl], start=True, stop=True)
        p_t = E.bp.tile([P, TQ], BF16, name="p_t", tag="p_t")
        nc.scalar.activation(p_t[:], ps_sT[:], EXP,
                             bias=0.0, scale=rk8[:, mc, hh:hh + 1])
        if E.debug and head == 0 and mc < 2:
            nc.sync.dma_start(E.dbg_p[:, mc * TQ:(mc + 1) * TQ], p_t[:])
        for half in range(2):
            hsl = slice(half * 512, (half + 1) * 512)
            nc.tensor.matmul(ps_ov[:, hsl], v_sb[:, mc, vidx, 0:65], p_t[:, hsl],
                             start=(mc == 0), stop=(mc == NMC - 1))
    # drain: unnormalized o + denominator reciprocal
    o_u = E.bo.tile([64, TQ], BF16, name="o_u")
    nc.scalar.copy(o_u[:], ps_ov[0:64, :])
    den = E.bo1.tile([1, TQ], F32, name="den")
    nc.vector.tensor_copy(den[:], ps_ov[64:65, :])
    rec = E.bo1.tile([1, TQ], F32, name="rec")
    nc.vector.reciprocal_approx_fast(out=rec[:], in_=den[:])
    rec_d = dram.tile([1, TQ], F32, name="rec_d")
    nc.sync.dma_start(rec_d[:], rec[:])
    if E.debug and head == 0:
        nc.sync.dma_start(E.dbg_ou[:], o_u[:])
        nc.sync.dma_start(E.dbg_rec[:], rec[:])
    rec_bc = E.bo1.tile([64, TQ], F32, name="rec_bc")
    nc.sync.dma_start(rec_bc[:], rec_d[:].to_broadcast((64, TQ)))
    nc.vector.tensor_tensor(
        E.o_fm[(head % 2) * 64:(head % 2) * 64 + 64, head // 2, :],
        o_u[:], rec_bc[:], MUL)


def _phase_b(E):
    nc, tc = E.nc, E.tc
    with (
        tc.tile_pool(name="bwv", bufs=1) as bwv,
        tc.tile_pool(name="bw", bufs=2) as bw,
        tc.tile_pool(name="bv", bufs=1) as bv,
        tc.tile_pool(name="bka", bufs=2) as bka,
        tc.tile_pool(name="bq", bufs=2) as bq,
        tc.tile_pool(name="bs1", bufs=1) as bs1,
        tc.tile_pool(name="bs0", bufs=1) as bs0,
        tc.tile_pool(name="bs2", bufs=2) as bs2,
        tc.tile_pool(name="bp", bufs=2) as bp,
        tc.tile_pool(name="bo", bufs=2) as bo,
        tc.tile_pool(name="bo1", bufs=1) as bo1,
        tc.tile_pool(name="ps", bufs=2, space="PSUM") as ps,
        tc.tile_pool(name="ps_big", bufs=2, space="PSUM") as ps_big,
        tc.tile_pool(name="ps_o", bufs=1, space="PSUM") as ps_o,
    ):
        E.bw, E.bka, E.bq = bw, bka, bq
        E.bs0, E.bs1, E.bs2 = bs0, bs1, bs2
        E.bp, E.bo, E.bo1 = bp, bo, bo1
        E.ps, E.ps_big, E.ps_o = ps, ps_big, ps_o
        for hq in range(4):  # head quads
            wv_sb = bwv.tile([P, CH, 256], F32R, name="wv_sb")
            nc.sync.dma_start(wv_sb[:], E.WprojT_r[:, :, 2 * C + hq * 256:2 * C + (hq + 1) * 256])
            v_sb = bv.tile([P, NMC, 4, 72], BF16, name="v_sb")
            nc.vector.memset(v_sb[:, :, :, 64:65], 1.0)
            for mtk in range(NMC):
                ps_v = ps.tile([P, 512], F32, name="ps_v", tag="ps")
                for ch in range(CH):
                    nc.tensor.matmul(ps_v[:, 0:256], E.h_sb[:, ch, mtk * P:(mtk + 1) * P],
                                     wv_sb[:, ch, :], start=(ch == 0), stop=(ch == CH - 1))
                # one strided copy: [128, 4 heads, 64], head stride 72 in dest
                nc.vector.tensor_copy(v_sb[:, mtk, :, 0:64],
                                      ps_v[:, 0:256].rearrange("p (h d) -> p h d", h=4))

            hp_data = []
            for hp in (2 * hq, 2 * hq + 1):
                wqk_sb, ka0, ka1, rk8 = _b_kproj(E, None, hp)
                qa0, qa1 = _b_qproj(E, wqk_sb)
                if E.debug and hp == 0:
                    nc.sync.dma_start(E.dbg_ka0[:], ka0[:])
                    nc.sync.dma_start(E.dbg_ka1[:], ka1[:])
                    nc.sync.dma_start(E.dbg_rk8[:], rk8[:].rearrange("p a b -> p (a b)"))
                    nc.sync.dma_start(E.dbg_qa0[:], qa0[:])
                    nc.sync.dma_start(E.dbg_qa1[:], qa1[:])
                hp_data.append((hp, (ka0, qa0), (ka1, qa1), rk8))

            for hp, h0, h1, rk8 in hp_data:
                for hh, (ka, qa) in enumerate((h0, h1)):
                    _b_scores(E, v_sb, hp, hh, ka, qa, rk8)


def _phase_c(E):
    nc, tc = E.nc, E.tc
    with (
        tc.tile_pool(name="cw2", bufs=1) as cw2,
        tc.tile_pool(name="ps_c", bufs=2, space="PSUM") as ps_c,
    ):
        for jt in range(C // 512):
            if jt == 0:
                wres_sb = E.wres0
            else:
                wres_sb = cw2.tile([P, CH, 512], BF16, name="wres_sb")
                nc.sync.dma_start(wres_sb[:], E.WresT_r[:, :, jt * 512:(jt + 1) * 512])
            for ns in range(TQ // P):
                ps_f = ps_c.tile([P, 512], F32, name="ps_f", tag="ps_c")
                for cg in range(CH):
                    nc.tensor.matmul(ps_f[:], E.o_fm[:, cg, ns * P:(ns + 1) * P],
                                     wres_sb[:, cg, :],
                                     start=(cg == 0), stop=(cg == CH - 1))
                f_sb = cw2.tile([P, 512], F32, name="f_sb")
                nc.scalar.copy(f_sb[:], ps_f[:])
                nc.sync.dma_start(E.out[ns * P:(ns + 1) * P, jt * 512:(jt + 1) * 512],
                                  f_sb[:])


def build_kernel(debug=False):
    import concourse.bacc as _bacc_mod
    _orig_tables = _bacc_mod.get_activation_tables

    def _one_set(arch):
        return {k: (v if k == "natural_log_exp_and_others" else frozenset())
                for k, v in _orig_tables(arch).items()}

    _bacc_mod.get_activation_tables = _one_set
    try:
        return _build_kernel_inner(debug)
    finally:
        _bacc_mod.get_activation_tables = _orig_tables


def _build_kernel_inner(debug=False):
    nc = bacc.Bacc()
    E = SimpleNamespace(nc=nc, debug=debug)

    E.xT = nc.dram_tensor("xT", [C, T], F32R, kind="ExternalInput")
    E.embT = nc.dram_tensor("embT", [EMB, T], BF16, kind="ExternalInput")
    E.WembT = nc.dram_tensor("WembT", [EMB, O2], BF16, kind="ExternalInput")
    E.bemb = nc.dram_tensor("bemb", [P, O2 // P], F32, kind="ExternalInput")
    E.WprojT = nc.dram_tensor("WprojT", [C, 3 * C], F32R, kind="ExternalInput")
    E.WresT = nc.dram_tensor("WresT", [C, C], BF16, kind="ExternalInput")
    E.out = nc.dram_tensor("out", [TQ, C], F32, kind="ExternalOutput")
    if debug:
        E.dbg_h = nc.dram_tensor("dbg_h", [P, CH, T], F32, kind="ExternalOutput")
        E.dbg_ka0 = nc.dram_tensor("dbg_ka0", [65, T], BF16, kind="ExternalOutput")
        E.dbg_ka1 = nc.dram_tensor("dbg_ka1", [65, T], BF16, kind="ExternalOutput")
        E.dbg_rk8 = nc.dram_tensor("dbg_rk8", [P, NMC * 2], F32, kind="ExternalOutput")
        E.dbg_qa0 = nc.dram_tensor("dbg_qa0", [65, TQ], BF16, kind="ExternalOutput")
        E.dbg_qa1 = nc.dram_tensor("dbg_qa1", [65, TQ], BF16, kind="ExternalOutput")
        E.dbg_p = nc.dram_tensor("dbg_p", [P, 2 * TQ], BF16, kind="ExternalOutput")
        E.dbg_ou = nc.dram_tensor("dbg_ou", [64, TQ], BF16, kind="ExternalOutput")
        E.dbg_rec = nc.dram_tensor("dbg_rec", [1, TQ], F32, kind="ExternalOutput")

    E.xT_r = E.xT.rearrange("(ch p) t -> p ch t", p=P)
    E.embT_r = E.embT.rearrange("(ch p) t -> p ch t", p=P)
    E.WembT_r = E.WembT.rearrange("(ch p) o -> p ch o", p=P)
    E.WprojT_r = E.WprojT.rearrange("(ch p) o -> p ch o", p=P)
    E.WresT_r = E.WresT.rearrange("(ch p) o -> p ch o", p=P)

    with tile.TileContext(nc) as tc:
        E.tc = tc
        with (
            tc.tile_pool(name="const", bufs=1) as const,
            tc.tile_pool(name="dram", bufs=2, space="DRAM") as dram,
        ):
            E.const, E.dram = const, dram
            _consts(E)
            with tc.tile_pool(name="hpool", bufs=1) as hpool:
                E.h_sb = hpool.tile([P, CH, T], F32R, name="h_sb")   # 64KB/part
                E.o_fm = hpool.tile([P, CH, TQ], BF16, name="o_fm")  # 16KB/part
                _phase_a1(E)
                with tc.tile_pool(name="cw", bufs=1) as cw:
                    E.cw = cw
                    E.wres0 = cw.tile([P, CH, 512], BF16, name="wres0")
                    nc.sync.dma_start(E.wres0[:], E.WresT_r[:, :, 0:512])
                    _phase_b(E)
                    _phase_c(E)

    nc.finalize()
    return nc


def _prep_host(x, emb, W_emb, b_emb, W_proj, W_out):
    BF = ml_dtypes.bfloat16
    W_embT = np.ascontiguousarray(W_emb.T).astype(BF)
    W_projT = np.ascontiguousarray(W_proj.T.astype(np.float32))
    W_resT = np.ascontiguousarray((np.eye(C, dtype=np.float32) + W_out).T).astype(BF)
    bemb2 = b_emb.astype(np.float32).copy()
    bemb2[:C] += 1.0                       # fold the FiLM "+1" into the bias
    bemb_col = np.ascontiguousarray(bemb2.reshape(O2 // P, P).T)

    in_maps = []
    for c in range(NCORES):
        b, j = c // 2, c % 2
        perm = np.concatenate([np.arange(j * TQ, (j + 1) * TQ),
                               np.arange((1 - j) * TQ, (2 - j) * TQ)])
        in_maps.append({
            "xT": np.ascontiguousarray(x[b][perm].T.astype(np.float32)),
            "embT": np.ascontiguousarray(emb[b][perm].T).astype(BF),
            "WembT": W_embT, "bemb": bemb_col,
            "WprojT": W_projT, "WresT": W_resT,
        })
    return in_maps


def kernel(x, emb, W_emb, b_emb, W_proj, W_out, _trace=False, _debug=False):
    x = np.asarray(x); emb = np.asarray(emb)
    W_emb = np.asarray(W_emb); b_emb = np.asarray(b_emb)
    W_proj = np.asarray(W_proj); W_out = np.asarray(W_out)

    key = "nc_dbg" if _debug else "nc"
    if key not in _cached:
        _cached[key] = build_kernel(debug=_debug)
    nc = _cached[key]

    in_maps = _prep_host(x, emb, W_emb, b_emb, W_proj, W_out)
    res = run_bass_kernel_spmd(nc, in_maps, core_ids=list(range(NCORES)), trace=_trace)
    _cached["last_result"] = res

    outp = np.empty((B, N, C), dtype=np.float32)
    for c in range(NCORES):
        b, j = c // 2, c % 2
        outp[b, j * TQ:(j + 1) * TQ, :] = res.results[c]["out"]
    _cached["last_out"] = outp
    return outp
